# revision 1
# baseline (speedup 1.0000x reference)
"""TRN2 Bass kernel for nn_CNNDSTv2_batch: out = mobius16(zeta16(M[:,0]) * zeta16(M[:,1])).

Math: the 16-bit superset-zeta factorizes as Z = A8 @ X @ A8^T on the 256x256
view X[hi_byte, lo_byte]; A8 = [[A7, A7], [0, A7]] block-triangular, so each
8-bit stage is 3 accumulating 128x128 matmuls reusing one stationary. Each
two-sided transform runs as [stage, transpose, stage] and yields the transposed
result; chaining zeta -> multiply -> mobius lands back in natural layout.

Precision: matmuls run in f32r (tf32-like, 11 explicit mantissa bits, 1
cyc/row). Stage-2 inputs are 2-term f32r hi/lo splits (~23 bits, exact since
the 0/+-1 stationaries are exact in f32r); the raw-input stage uses a single
rounding, which commutes with the positive-sum conjunction and is safe. The
splits happen at the stage-1 PSUM exit; the PE transposes then move the
already-rounded planes (the ldw-opt transpose path rounds its stationary to
f32r, which is lossless on pre-rounded data).

Perf: walrus's disabled enable-ldw-opt pass is turned on (run_command shim) so
weight loads are separated from matmuls and overlap them (2x matmul issue
rate). It miscompiles nothing we emit: regular matmuls and f32r transposes are
verified bit-correct under it; fp32 is_transpose (which it breaks) is avoided.

Sharding: pure data parallel, batch 512 -> 64 per core across 8 cores.
"""
import sys
import os
import functools

sys.path.insert(0, "/opt/trn_rl_repo")
import numpy as np

BATCH = 512
L = 65536
NCORES = 8
BPC = BATCH // NCORES          # 64 batch elems per core
PAIRS = BPC // 2               # 2 elems per pipeline iteration


def _pc(v):
    return bin(v).count("1")


def _constants():
    k = np.arange(128)
    sup = (k[:, None] & k[None, :]) == k[None, :]          # sup[k,m] = k superset of m
    AT7 = sup.astype(np.float32)                           # lhsT for A7 @ x
    pc = np.array([_pc(i) for i in range(128)])
    sign = (-1.0) ** (pc[:, None] - pc[None, :])
    BT7 = (sup * sign).astype(np.float32)                  # lhsT for B7 @ x
    return AT7, BT7


def _patch_ldw_opt():
    import concourse.bass_utils as bu
    if getattr(bu, "_ldw_opt_patched", False):
        return
    orig = bu.run_command

    def patched(argv, **kw):
        argv = [a.replace("--enable-ldw-opt=false", "--enable-ldw-opt=true")
                for a in argv]
        return orig(argv, **kw)

    bu.run_command = patched
    bu._ldw_opt_patched = True


def _build():
    import concourse.bacc as bacc
    import concourse.tile as tile
    import concourse.mybir as mybir

    _patch_ldw_opt()

    dt = mybir.dt
    F32, F32R = dt.float32, dt.float32r

    nc = bacc.Bacc("TRN2", target_bir_lowering=False, debug=False)

    # HBM layout (host pre-permuted, all DMAs contiguous):
    # Mi[pair, ch, p(=bits14..8), (b, I=bit15, J=bit7, l=bits6..0)]
    Mi = nc.dram_tensor("Mi", [PAIRS, 2, 128, 1024], F32R, kind="ExternalInput").ap()
    # C = [AT7 | BT7 | -BT7] as f32r (exact 0/+-1), IdR = f32r identity
    C = nc.dram_tensor("C", [128, 384], F32R, kind="ExternalInput").ap()
    Id_d = nc.dram_tensor("Id", [128, 128], F32R, kind="ExternalInput").ap()
    # O[pair, p, (I''=bit15, b, J=bit7, l=bits6..0)] - host unscrambles
    O = nc.dram_tensor("O", [PAIRS, 128, 1024], F32, kind="ExternalOutput").ap()

    with tile.TileContext(nc) as tc:
        with tc.tile_pool(name="const", bufs=1) as cp, \
             tc.tile_pool(name="sbuf", bufs=2) as sb, \
             tc.tile_pool(name="psA", bufs=4, space="PSUM") as psA:
            psB = psA
            Ct = cp.tile([128, 384], F32R, tag="C")
            nc.sync.dma_start(Ct[:], C)
            IdR = cp.tile([128, 128], F32R, tag="IdR")
            nc.sync.dma_start(IdR[:], Id_d)
            AT = Ct[:, 0:128]
            BT = Ct[:, 128:256]
            nBT = Ct[:, 256:384]

            def mm(out_ap, lhsT, rhs, start, stop):
                nc.tensor.matmul(out_ap, lhsT, rhs, start=start, stop=stop)

            def stage(dst, M, Mn, s0, s1):
                """dst[:, :512] = M@s0 + Mn@s1 ; dst[:, 512:] = M@s1.
                s0/s1: lists of 1-2 moving APs (f32r planes)."""
                d1 = dst[:, 512:1024]
                for i, a in enumerate(s1):
                    mm(d1, M, a, start=(i == 0), stop=(i == len(s1) - 1))
                d0 = dst[:, 0:512]
                for i, a in enumerate(s0):
                    mm(d0, M, a, start=(i == 0), stop=False)
                for i, a in enumerate(s1):
                    mm(d0, Mn, a, start=False, stop=(i == len(s1) - 1))

            def transpose_plane(dst, src):
                """dst[:, Jd*512 + b*256 + K*128 +: 128] =
                   src[:, K*512 + b*256 + Jd*128 +: 128].T  for Jd,b,K in {0,1}.
                f32r is_transpose (1.5 cyc/row); under ldw-opt the stationary is
                rounded to f32r, a no-op on pre-rounded planes. One start/stop
                group per 512-wide PSUM bank."""
                for Jd in (0, 1):
                    k = 0
                    for b in (0, 1):
                        for K in (0, 1):
                            nc.tensor.matmul(
                                dst[:, Jd * 512 + b * 256 + K * 128:][:, :128],
                                src[:, K * 512 + b * 256 + Jd * 128:][:, :128],
                                IdR[:], is_transpose=True,
                                start=(k == 0), stop=(k == 3))
                            k += 1

            # --- software-pipelined emission: 2 pairs interleaved ---
            st = {}

            def dma_in(pr, c):
                xin = sb.tile([128, 1024], F32R, tag=f"xin{c}", bufs=4,
                              name=f"xin{c}")
                nc.sync.dma_start(xin[:], Mi[pr, c])
                st[pr, c, "x"] = xin

            def zeta_s1(pr, c):
                xr = st[pr, c, "x"][:].rearrange("p (b i f) -> p b i f", b=2, i=2)
                y = psA.tile([128, 1024], F32, tag="a", name="y")
                stage(y[:], AT, AT, [xr[:, :, 0]], [xr[:, :, 1]])
                st[pr, c, "y"] = y

            def split_y(pr, c):
                y = st[pr, c, "y"]
                yh = sb.tile([128, 1024], F32R, tag=f"yh{c}", name=f"yh{c}")
                nc.scalar.copy(yh[:], y[:])
                yl = sb.tile([128, 1024], F32R, tag=f"yl{c}", name=f"yl{c}")
                nc.vector.tensor_sub(yl[:], y[:], yh[:].bitcast(F32))
                st[pr, c, "hl"] = (yh, yl)

            def trans_y(pr, c):
                yh, yl = st[pr, c, "hl"]
                yhT = psB.tile([128, 1024], F32R, tag="a", name="yhT")
                transpose_plane(yhT[:], yh[:])
                ylT = psB.tile([128, 1024], F32R, tag="a", name="ylT")
                transpose_plane(ylT[:], yl[:])
                st[pr, c, "T"] = (yhT, ylT)

            def copy_T(pr, c):
                yhT, ylT = st[pr, c, "T"]
                yhTs = sb.tile([128, 1024], F32R, tag=f"yhTs{c}", name=f"yhTs{c}", bufs=3)
                nc.scalar.copy(yhTs[:], yhT[:])
                ylTs = sb.tile([128, 1024], F32R, tag=f"ylTs{c}", name=f"ylTs{c}", bufs=3)
                nc.vector.tensor_copy(ylTs[:], ylT[:])
                st[pr, c, "Ts"] = (yhTs, ylTs)

            def zeta_s2(pr, c):
                yhTs, ylTs = st[pr, c, "Ts"]
                z = psA.tile([128, 1024], F32, tag="a", name="z")
                stage(z[:], AT, AT,
                      [yhTs[:, 0:512], ylTs[:, 0:512]],
                      [yhTs[:, 512:1024], ylTs[:, 512:1024]])
                if c == 0:
                    z0s = sb.tile([128, 1024], F32, tag="z0s", name="z0s")
                    nc.scalar.copy(z0s[:], z[:])
                    st[pr, "z0s"] = z0s
                else:
                    t = sb.tile([128, 1024], F32, tag="t", name="t")
                    nc.vector.tensor_mul(t[:], z[:], st[pr, "z0s"][:])
                    qh = sb.tile([128, 1024], F32R, tag="qh", name="qh", bufs=3)
                    nc.gpsimd.tensor_copy(qh[:], t[:])
                    ql = sb.tile([128, 1024], F32R, tag="ql", name="ql", bufs=3)
                    nc.gpsimd.tensor_sub(ql[:], t[:], qh[:].bitcast(F32))
                    st[pr, "q"] = (qh, ql)

            def mob_s1(pr):
                qh, ql = st[pr, "q"]
                u = psA.tile([128, 1024], F32, tag="a", name="u")
                stage(u[:], BT, nBT,
                      [qh[:, 0:512], ql[:, 0:512]],
                      [qh[:, 512:1024], ql[:, 512:1024]])
                st[pr, "u"] = u

            def split_u(pr):
                u = st[pr, "u"]
                uh = sb.tile([128, 1024], F32R, tag="uh", name="uh")
                nc.scalar.copy(uh[:], u[:])
                ul = sb.tile([128, 1024], F32R, tag="ul", name="ul")
                nc.vector.tensor_sub(ul[:], u[:], uh[:].bitcast(F32))
                st[pr, "uhl"] = (uh, ul)

            def trans_u(pr):
                uh, ul = st[pr, "uhl"]
                uhT = psB.tile([128, 1024], F32R, tag="a", name="uhT")
                transpose_plane(uhT[:], uh[:])
                ulT = psB.tile([128, 1024], F32R, tag="a", name="ulT")
                transpose_plane(ulT[:], ul[:])
                st[pr, "uT"] = (uhT, ulT)

            def copy_uT(pr):
                uhT, ulT = st[pr, "uT"]
                uhTs = sb.tile([128, 1024], F32R, tag="uhTs", name="uhTs", bufs=3)
                nc.scalar.copy(uhTs[:], uhT[:])
                ulTs = sb.tile([128, 1024], F32R, tag="ulTs", name="ulTs", bufs=3)
                nc.vector.tensor_copy(ulTs[:], ulT[:])
                st[pr, "uTs"] = (uhTs, ulTs)

            def mob_s2(pr):
                uhTs, ulTs = st[pr, "uTs"]
                o = psA.tile([128, 1024], F32, tag="a", name="o")
                stage(o[:], BT, nBT,
                      [uhTs[:, 0:512], ulTs[:, 0:512]],
                      [uhTs[:, 512:1024], ulTs[:, 512:1024]])
                osb = sb.tile([128, 1024], F32, tag="osb", name="osb")
                nc.scalar.copy(osb[:], o[:])
                nc.sync.dma_start(O[pr], osb[:])

            G = 2
            for g in range(0, PAIRS, G):
                prs = range(g, min(g + G, PAIRS))
                for pr in prs:
                    dma_in(pr, 0)
                    dma_in(pr, 1)
                for c in (0, 1):
                    for pr in prs:
                        zeta_s1(pr, c)
                    for pr in prs:
                        split_y(pr, c)
                    for pr in prs:
                        trans_y(pr, c)
                    for pr in prs:
                        copy_T(pr, c)
                    for pr in prs:
                        zeta_s2(pr, c)
                for pr in prs:
                    mob_s1(pr)
                for pr in prs:
                    split_u(pr)
                for pr in prs:
                    trans_u(pr)
                for pr in prs:
                    copy_uT(pr)
                for pr in prs:
                    mob_s2(pr)

    nc.compile()
    return nc


@functools.lru_cache(maxsize=1)
def _get_nc():
    return _build()


def _host_in(M):
    """M [512, 2, 65536] f32 -> per-core Mi [PAIRS, 2, 128, 1024] contiguous.
    index16 = I*2^15 + p*2^8 + J*2^7 + l ; f-order (b, I, J, l)."""
    M6 = np.asarray(M, dtype=np.float32).reshape(NCORES, PAIRS, 2, 2, 2, 128, 2, 128)
    #                                      core, pair, b,  ch, I,  p,   J,  l
    Mi = np.ascontiguousarray(M6.transpose(0, 1, 3, 5, 2, 4, 6, 7))
    #                                      core, pair, ch, p, b, I, J, l
    return Mi.reshape(NCORES, PAIRS, 2, 128, 1024)


def _host_out(Os):
    """Os list of [PAIRS, 128, 1024] per core -> [512, 65536, 1, 1].
    o f-layout (I'', b, J, l)."""
    O = np.stack(Os).reshape(NCORES, PAIRS, 128, 2, 2, 2, 128)
    #                         core, pair, p, I, b, J, l
    out = np.ascontiguousarray(O.transpose(0, 1, 4, 3, 2, 5, 6))
    #                                      core, pair, b, I, p, J, l
    return out.reshape(BATCH, L, 1, 1)


def _run(M, trace=False):
    from concourse.bass_utils import run_bass_kernel_spmd
    nc = _get_nc()
    AT7, BT7 = _constants()
    C = np.concatenate([AT7, BT7, -BT7], axis=1)
    Id = np.eye(128, dtype=np.float32)
    Mi = _host_in(M)
    in_maps = [{"Mi": Mi[k], "C": C, "Id": Id} for k in range(NCORES)]
    res = run_bass_kernel_spmd(nc, in_maps, list(range(NCORES)), trace=trace)
    out = _host_out([res.results[k]["O"] for k in range(NCORES)])
    return out, res


def kernel(M):
    try:
        out, _ = _run(M, trace=False)
    except Exception:
        # one retry: a cold first execute has been observed to flake
        # (NRT_EXEC_UNIT_UNRECOVERABLE) and recover on rerun
        out, _ = _run(M, trace=False)
    return out



# revision 9
# speedup vs baseline: 1.8494x; 1.8494x over previous
"""TRN2 Bass kernel for nn_CNNDSTv2_batch: out = mobius16(zeta16(M[:,0]) * zeta16(M[:,1])).

Math: the 16-bit superset-zeta factorizes as Z = A8 @ X @ A8^T on the 256x256
view X[hi_byte, lo_byte]; A8 = [[A7, A7], [0, A7]] block-triangular, so each
8-bit stage is 3 accumulating 128x128 matmuls reusing one stationary. Each
two-sided transform runs as [stage, transpose, stage] and yields the transposed
result; chaining zeta -> multiply -> mobius lands back in natural layout.

Precision (modeled in numpy against f64, gate 2e-2, model says 3.9e-3):
- zeta path runs fully in bf16 (inputs are positive and the zeta->product->
  mobius composition is a positive map, so input/mid roundings stay relative);
- q (the commonality product feeding Mobius) keeps a 2-term f32r hi/lo split -
  rounding q is amplified ~100x by Mobius cancellation and dominates the
  error budget if single-rounded;
- the mobius mid-plane u is single-rounded f32r; the output is written bf16
  (final rounding, unamplified).

Perf: walrus's disabled enable-ldw-opt pass is turned on (run_command shim) so
weight loads overlap matmuls. bf16 halves input DMA, y-plane copy bytes (2x
DVE rate) and transpose cost (1.0 cyc/row), and bf16 transpose PSUM tiles are
1 bank. Copies are balanced across Activation and DVE; nothing sits on the
slow Pool engine.

Sharding: pure data parallel, batch 512 -> 64 per core across 8 cores.
"""
import sys
import os
import functools

sys.path.insert(0, "/opt/trn_rl_repo")
import numpy as np

BATCH = 512
L = 65536
NCORES = 8
BPC = BATCH // NCORES          # 64 batch elems per core
PAIRS = BPC // 2               # 2 elems per pipeline iteration


def _pc(v):
    return bin(v).count("1")


def _constants():
    k = np.arange(128)
    sup = (k[:, None] & k[None, :]) == k[None, :]          # sup[k,m] = k superset of m
    AT7 = sup.astype(np.float32)                           # lhsT for A7 @ x
    pc = np.array([_pc(i) for i in range(128)])
    sign = (-1.0) ** (pc[:, None] - pc[None, :])
    BT7 = (sup * sign).astype(np.float32)                  # lhsT for B7 @ x
    return AT7, BT7


def _build():
    import concourse.bacc as bacc
    import concourse.tile as tile
    import concourse.mybir as mybir

    # NOTE: walrus --enable-ldw-opt stays OFF: it miscompiles bf16 (and fp32)
    # is_transpose matmuls (verified: transposed planes come out as garbage).
    # The tile layer already pre-splits 2-byte-dtype matmuls into hoisted
    # InstLdweights + matmul, so bf16 weight loads overlap anyway; only the
    # f32r mobius matmuls pay a serial self-load.
    dt = mybir.dt
    F32, F32R, BF16 = dt.float32, dt.float32r, dt.bfloat16

    nc = bacc.Bacc("TRN2", target_bir_lowering=False, debug=False)

    # HBM layout (host pre-permuted, all DMAs contiguous):
    # Mi[pair, ch, p(=bits14..8), (b, I=bit15, J=bit7, l=bits6..0)] in bf16
    Mi = nc.dram_tensor("Mi", [PAIRS, 2, 128, 1024], BF16, kind="ExternalInput").ap()
    # Cb = [AT7 | Id] bf16 (exact 0/1), Cr = [BT7 | -BT7 | Id] f32r (exact 0/+-1)
    Cb_d = nc.dram_tensor("Cb", [128, 256], BF16, kind="ExternalInput").ap()
    Cr_d = nc.dram_tensor("Cr", [128, 384], F32R, kind="ExternalInput").ap()
    # O[pair, p, (I''=bit15, b, J=bit7, l=bits6..0)] bf16 - host unscrambles
    O = nc.dram_tensor("O", [PAIRS, 128, 1024], BF16, kind="ExternalOutput").ap()

    with tile.TileContext(nc) as tc:
        with tc.tile_pool(name="const", bufs=1) as cp, \
             tc.tile_pool(name="sbuf", bufs=2) as sb, \
             tc.tile_pool(name="psA", bufs=3, space="PSUM") as psA:
            Cb = cp.tile([128, 256], BF16, tag="Cb")
            nc.sync.dma_start(Cb[:], Cb_d)
            Cr = cp.tile([128, 384], F32R, tag="Cr")
            nc.sync.dma_start(Cr[:], Cr_d)
            ATb = Cb[:, 0:128]
            Idb = Cb[:, 128:256]
            BT = Cr[:, 0:128]
            nBT = Cr[:, 128:256]
            IdR = Cr[:, 256:384]

            def mm(out_ap, lhsT, rhs, start, stop):
                nc.tensor.matmul(out_ap, lhsT, rhs, start=start, stop=stop)

            def stage(dst, M, Mn, s0, s1):
                """dst[:, :512] = M@s0 + Mn@s1 ; dst[:, 512:] = M@s1.
                s0/s1: lists of 1-2 moving APs."""
                d1 = dst[:, 512:1024]
                for i, a in enumerate(s1):
                    mm(d1, M, a, start=(i == 0), stop=(i == len(s1) - 1))
                d0 = dst[:, 0:512]
                for i, a in enumerate(s0):
                    mm(d0, M, a, start=(i == 0), stop=False)
                for i, a in enumerate(s1):
                    mm(d0, Mn, a, start=False, stop=(i == len(s1) - 1))

            def transpose_plane(dst, src, Id):
                """dst[:, Jd*512 + b*256 + K*128 +: 128] =
                   src[:, K*512 + b*256 + Jd*128 +: 128].T  for Jd,b,K in {0,1}.
                One start/stop group per 512-wide half."""
                for Jd in (0, 1):
                    k = 0
                    for b in (0, 1):
                        for K in (0, 1):
                            nc.tensor.matmul(
                                dst[:, Jd * 512 + b * 256 + K * 128:][:, :128],
                                src[:, K * 512 + b * 256 + Jd * 128:][:, :128],
                                Id, is_transpose=True,
                                start=(k == 0), stop=(k == 3))
                            k += 1

            # --- software-pipelined emission: 2 pairs interleaved ---
            st = {}

            def dma_in(pr, c):
                xin = sb.tile([128, 1024], BF16, tag=f"xin{c}", bufs=4,
                              name=f"xin{c}")
                nc.sync.dma_start(xin[:], Mi[pr, c])
                st[pr, c, "x"] = xin

            def zeta_s1(pr, c):
                xr = st[pr, c, "x"][:].rearrange("p (b i f) -> p b i f", b=2, i=2)
                y = psA.tile([128, 1024], F32, tag="a", name="y")
                stage(y[:], ATb, ATb, [xr[:, :, 0]], [xr[:, :, 1]])
                st[pr, c, "y"] = y

            def copy_ys(pr, c):
                # PSUM f32 -> SBUF bf16, single rounding on the zeta path
                y = st[pr, c, "y"]
                ys = sb.tile([128, 1024], BF16, tag=f"ys{c}", name=f"ys{c}", bufs=3)
                nc.scalar.copy(ys[:], y[:])
                st[pr, c, "ys"] = ys

            def trans_y(pr, c):
                yT = psA.tile([128, 1024], BF16, tag="pT", bufs=2, name="yT")
                transpose_plane(yT[:], st[pr, c, "ys"][:], Idb)
                st[pr, c, "yT"] = yT

            def copy_yTs(pr, c):
                yTs = sb.tile([128, 1024], BF16, tag=f"yTs{c}", name=f"yTs{c}", bufs=3)
                nc.vector.tensor_copy(yTs[:], st[pr, c, "yT"][:])
                st[pr, c, "Ts"] = yTs

            def zeta_s2(pr, c):
                yTs = st[pr, c, "Ts"]
                z = psA.tile([128, 1024], F32, tag="a", name="z")
                stage(z[:], ATb, ATb, [yTs[:, 0:512]], [yTs[:, 512:1024]])
                if c == 0:
                    z0s = sb.tile([128, 1024], F32, tag="z0s", name="z0s")
                    nc.scalar.copy(z0s[:], z[:])
                    st[pr, "z0s"] = z0s
                else:
                    qf = sb.tile([128, 1024], F32, tag="qf", name="qf")
                    nc.vector.tensor_mul(qf[:], z[:], st[pr, "z0s"][:])
                    qh = sb.tile([128, 1024], F32R, tag="qh", name="qh", bufs=3)
                    nc.scalar.copy(qh[:], qf[:])
                    ql = sb.tile([128, 1024], F32R, tag="ql", name="ql", bufs=3)
                    nc.vector.tensor_sub(ql[:], qf[:], qh[:].bitcast(F32))
                    st[pr, "q"] = (qh, ql)

            def mob_s1(pr):
                qh, ql = st[pr, "q"]
                u = psA.tile([128, 1024], F32, tag="a", name="u")
                stage(u[:], BT, nBT,
                      [qh[:, 0:512], ql[:, 0:512]],
                      [qh[:, 512:1024], ql[:, 512:1024]])
                st[pr, "u"] = u

            def copy_us(pr):
                us = sb.tile([128, 1024], F32R, tag="us", name="us")
                nc.scalar.copy(us[:], st[pr, "u"][:])
                st[pr, "us"] = us

            def trans_u(pr):
                uT = psA.tile([128, 1024], F32R, tag="a", name="uT")
                transpose_plane(uT[:], st[pr, "us"][:], IdR)
                st[pr, "uT"] = uT

            def copy_uTs(pr):
                uTs = sb.tile([128, 1024], F32R, tag="uTs", name="uTs")
                nc.vector.tensor_copy(uTs[:], st[pr, "uT"][:])
                st[pr, "uTs"] = uTs

            def mob_s2(pr):
                uTs = st[pr, "uTs"]
                o = psA.tile([128, 1024], F32, tag="a", name="o")
                stage(o[:], BT, nBT, [uTs[:, 0:512]], [uTs[:, 512:1024]])
                osb = sb.tile([128, 1024], BF16, tag="osb", name="osb")
                nc.vector.tensor_copy(osb[:], o[:])
                nc.sync.dma_start(O[pr], osb[:])

            G = 2
            for g in range(0, PAIRS, G):
                prs = range(g, min(g + G, PAIRS))
                for pr in prs:
                    dma_in(pr, 0)
                    dma_in(pr, 1)
                for c in (0, 1):
                    for pr in prs:
                        zeta_s1(pr, c)
                    for pr in prs:
                        copy_ys(pr, c)
                    for pr in prs:
                        trans_y(pr, c)
                    for pr in prs:
                        copy_yTs(pr, c)
                    for pr in prs:
                        zeta_s2(pr, c)
                for pr in prs:
                    mob_s1(pr)
                for pr in prs:
                    copy_us(pr)
                for pr in prs:
                    trans_u(pr)
                for pr in prs:
                    copy_uTs(pr)
                for pr in prs:
                    mob_s2(pr)

    nc.compile()
    return nc


@functools.lru_cache(maxsize=1)
def _get_nc():
    return _build()


def _host_in(M):
    """M [512, 2, 65536] f32 -> per-core Mi [PAIRS, 2, 128, 1024] bf16 contiguous.
    index16 = I*2^15 + p*2^8 + J*2^7 + l ; f-order (b, I, J, l)."""
    import ml_dtypes
    M6 = np.asarray(M, dtype=np.float32).reshape(NCORES, PAIRS, 2, 2, 2, 128, 2, 128)
    #                                      core, pair, b,  ch, I,  p,   J,  l
    Mi = np.ascontiguousarray(M6.transpose(0, 1, 3, 5, 2, 4, 6, 7))
    #                                      core, pair, ch, p, b, I, J, l
    return Mi.reshape(NCORES, PAIRS, 2, 128, 1024).astype(ml_dtypes.bfloat16)


def _host_out(Os):
    """Os list of [PAIRS, 128, 1024] bf16 per core -> [512, 65536, 1, 1] f32.
    o f-layout (I'', b, J, l)."""
    O = np.stack(Os).astype(np.float32).reshape(NCORES, PAIRS, 128, 2, 2, 2, 128)
    #                                            core, pair, p, I, b, J, l
    out = np.ascontiguousarray(O.transpose(0, 1, 4, 3, 2, 5, 6))
    #                                      core, pair, b, I, p, J, l
    return out.reshape(BATCH, L, 1, 1)


def _run(M, trace=False):
    import ml_dtypes
    from concourse.bass_utils import run_bass_kernel_spmd
    nc = _get_nc()
    AT7, BT7 = _constants()
    Cb = np.concatenate([AT7, np.eye(128, dtype=np.float32)],
                        axis=1).astype(ml_dtypes.bfloat16)
    Cr = np.concatenate([BT7, -BT7, np.eye(128, dtype=np.float32)], axis=1)
    Mi = _host_in(M)
    in_maps = [{"Mi": Mi[k], "Cb": Cb, "Cr": Cr} for k in range(NCORES)]
    res = run_bass_kernel_spmd(nc, in_maps, list(range(NCORES)), trace=trace)
    out = _host_out([res.results[k]["O"] for k in range(NCORES)])
    return out, res


def kernel(M):
    try:
        out, _ = _run(M, trace=False)
    except Exception:
        # one retry: a cold first execute has been observed to flake
        # (NRT_EXEC_UNIT_UNRECOVERABLE) and recover on rerun
        out, _ = _run(M, trace=False)
    return out


# revision 14
# speedup vs baseline: 2.0879x; 1.1290x over previous
"""TRN2 Bass kernel for nn_CNNDSTv2_batch: out = mobius16(zeta16(M[:,0]) * zeta16(M[:,1])).

Math: the 16-bit superset-zeta factorizes as Z = A8 @ X @ A8^T on the 256x256
view X[hi_byte, lo_byte]; A8 = [[A7, A7], [0, A7]] block-triangular, so each
8-bit stage is 3 accumulating 128x128 matmuls reusing one stationary. Each
two-sided transform runs as [stage, transpose, stage] and yields the transposed
result; chaining zeta -> multiply -> mobius lands back in natural layout.

Precision (modeled in numpy against f64, gate 2e-2, model says 3.9e-3):
- zeta path runs fully in bf16 (inputs are positive and the zeta->product->
  mobius composition is a positive map, so input/mid roundings stay relative);
- q (the commonality product feeding Mobius) keeps a 2-term f32r hi/lo split -
  rounding q is amplified ~100x by Mobius cancellation and dominates the
  error budget if single-rounded;
- the mobius mid-plane u is single-rounded f32r; the output is written bf16
  (final rounding, unamplified).

Perf: bf16 halves input DMA, y-plane copy bytes (2x DVE rate) and transpose
cost (1.0 cyc/row), and bf16 transpose PSUM tiles are 1 bank. walrus
enable-ldw-opt stays OFF (it miscompiles bf16/fp32 is_transpose); the tile
layer's own hoisted InstLdweights split gives overlapped weight loads anyway.
Emission is a flat skewed software pipeline (one pair per slot, 18 stage
positions) so the PE never waits on a same-pair dependency chain and holds
its 2.4 GHz p-state; copies are balanced across Activation/DVE/Pool.

Sharding: pure data parallel, batch 512 -> 64 per core across 8 cores.
"""
import sys
import os
import functools

sys.path.insert(0, "/opt/trn_rl_repo")
import numpy as np

BATCH = 512
L = 65536
NCORES = 8
BPC = BATCH // NCORES          # 64 batch elems per core
PAIRS = BPC // 2               # 2 elems per pipeline iteration


def _pc(v):
    return bin(v).count("1")


def _constants():
    k = np.arange(128)
    sup = (k[:, None] & k[None, :]) == k[None, :]          # sup[k,m] = k superset of m
    AT7 = sup.astype(np.float32)                           # lhsT for A7 @ x
    pc = np.array([_pc(i) for i in range(128)])
    sign = (-1.0) ** (pc[:, None] - pc[None, :])
    BT7 = (sup * sign).astype(np.float32)                  # lhsT for B7 @ x
    return AT7, BT7


def _build():
    import concourse.bacc as bacc
    import concourse.tile as tile
    import concourse.mybir as mybir

    # NOTE: walrus --enable-ldw-opt stays OFF: it miscompiles bf16 (and fp32)
    # is_transpose matmuls (verified: transposed planes come out as garbage).
    # The tile layer already pre-splits 2-byte-dtype matmuls into hoisted
    # InstLdweights + matmul, so bf16 weight loads overlap anyway; only the
    # f32r mobius matmuls pay a serial self-load.
    dt = mybir.dt
    F32, F32R, BF16 = dt.float32, dt.float32r, dt.bfloat16

    nc = bacc.Bacc("TRN2", target_bir_lowering=False, debug=False)

    # HBM layout (host pre-permuted, all DMAs contiguous):
    # Mi[pair, ch, p(=bits14..8), (b, I=bit15, J=bit7, l=bits6..0)] in bf16
    Mi = nc.dram_tensor("Mi", [PAIRS, 2, 128, 1024], BF16, kind="ExternalInput").ap()
    # Cb = [AT7 | Id] bf16 (exact 0/1), Cr = [BT7 | -BT7 | Id] f32r (exact 0/+-1)
    Cb_d = nc.dram_tensor("Cb", [128, 256], BF16, kind="ExternalInput").ap()
    Cr_d = nc.dram_tensor("Cr", [128, 384], F32R, kind="ExternalInput").ap()
    # O[pair, p, (I''=bit15, b, J=bit7, l=bits6..0)] bf16 - host unscrambles
    O = nc.dram_tensor("O", [PAIRS, 128, 1024], BF16, kind="ExternalOutput").ap()

    with tile.TileContext(nc) as tc:
        with tc.tile_pool(name="const", bufs=1) as cp, \
             tc.tile_pool(name="sbuf", bufs=2) as sb, \
             tc.tile_pool(name="psA", bufs=3, space="PSUM") as psA:
            Cb = cp.tile([128, 256], BF16, tag="Cb")
            nc.sync.dma_start(Cb[:], Cb_d)
            Cr = cp.tile([128, 384], F32R, tag="Cr")
            nc.sync.dma_start(Cr[:], Cr_d)
            ATb = Cb[:, 0:128]
            Idb = Cb[:, 128:256]
            BT = Cr[:, 0:128]
            nBT = Cr[:, 128:256]
            IdR = Cr[:, 256:384]

            def mm(out_ap, lhsT, rhs, start, stop):
                nc.tensor.matmul(out_ap, lhsT, rhs, start=start, stop=stop)

            def stage(dst, M, Mn, s0, s1):
                """dst[:, :512] = M@s0 + Mn@s1 ; dst[:, 512:] = M@s1.
                s0/s1: lists of 1-2 moving APs."""
                d1 = dst[:, 512:1024]
                for i, a in enumerate(s1):
                    mm(d1, M, a, start=(i == 0), stop=(i == len(s1) - 1))
                d0 = dst[:, 0:512]
                for i, a in enumerate(s0):
                    mm(d0, M, a, start=(i == 0), stop=False)
                for i, a in enumerate(s1):
                    mm(d0, Mn, a, start=False, stop=(i == len(s1) - 1))

            def transpose_plane(dst, src, Id):
                """dst[:, Jd*512 + b*256 + K*128 +: 128] =
                   src[:, K*512 + b*256 + Jd*128 +: 128].T  for Jd,b,K in {0,1}.
                One start/stop group per 512-wide half."""
                for Jd in (0, 1):
                    k = 0
                    for b in (0, 1):
                        for K in (0, 1):
                            nc.tensor.matmul(
                                dst[:, Jd * 512 + b * 256 + K * 128:][:, :128],
                                src[:, K * 512 + b * 256 + Jd * 128:][:, :128],
                                Id, is_transpose=True,
                                start=(k == 0), stop=(k == 3))
                            k += 1

            # --- software-pipelined emission: 2 pairs interleaved ---
            st = {}

            def dma_in(pr, c):
                xin = sb.tile([128, 1024], BF16, tag=f"xin{c}", bufs=4,
                              name=f"xin{c}")
                nc.sync.dma_start(xin[:], Mi[pr, c])
                st[pr, c, "x"] = xin

            def zeta_s1(pr, c):
                xr = st[pr, c, "x"][:].rearrange("p (b i f) -> p b i f", b=2, i=2)
                y = psA.tile([128, 1024], F32, tag="a", name="y")
                stage(y[:], ATb, ATb, [xr[:, :, 0]], [xr[:, :, 1]])
                st[pr, c, "y"] = y

            def copy_ys(pr, c):
                # PSUM f32 -> SBUF bf16, single rounding on the zeta path
                y = st[pr, c, "y"]
                ys = sb.tile([128, 1024], BF16, tag=f"ys{c}", name=f"ys{c}", bufs=3)
                nc.scalar.copy(ys[:], y[:])
                st[pr, c, "ys"] = ys

            def trans_y(pr, c):
                yT = psA.tile([128, 1024], BF16, tag="pT", bufs=2, name="yT")
                transpose_plane(yT[:], st[pr, c, "ys"][:], Idb)
                st[pr, c, "yT"] = yT

            def copy_yTs(pr, c):
                yTs = sb.tile([128, 1024], BF16, tag=f"yTs{c}", name=f"yTs{c}", bufs=3)
                nc.vector.tensor_copy(yTs[:], st[pr, c, "yT"][:])
                st[pr, c, "Ts"] = yTs

            def zeta_s2(pr, c):
                yTs = st[pr, c, "Ts"]
                z = psA.tile([128, 1024], F32, tag="a", name="z")
                stage(z[:], ATb, ATb, [yTs[:, 0:512]], [yTs[:, 512:1024]])
                if c == 0:
                    # z0s lives pos7 -> pos12 in the skew: deep ring
                    z0s = sb.tile([128, 1024], F32, tag="z0s", name="z0s", bufs=7)
                    nc.scalar.copy(z0s[:], z[:])
                    st[pr, "z0s"] = z0s
                else:
                    qf = sb.tile([128, 1024], F32, tag="qf", name="qf", bufs=3)
                    nc.vector.tensor_mul(qf[:], z[:], st[pr, "z0s"][:])
                    # q hi/lo split: hi on the (otherwise idle) Pool engine,
                    # lo on DVE; cross-pair skew hides Pool's slowness
                    qh = sb.tile([128, 1024], F32R, tag="qh", name="qh", bufs=3)
                    nc.gpsimd.tensor_copy(qh[:], qf[:])
                    ql = sb.tile([128, 1024], F32R, tag="ql", name="ql", bufs=3)
                    nc.vector.tensor_sub(ql[:], qf[:], qh[:].bitcast(F32))
                    st[pr, "q"] = (qh, ql)

            def mob_s1(pr):
                qh, ql = st[pr, "q"]
                u = psA.tile([128, 1024], F32, tag="a", name="u")
                stage(u[:], BT, nBT,
                      [qh[:, 0:512], ql[:, 0:512]],
                      [qh[:, 512:1024], ql[:, 512:1024]])
                st[pr, "u"] = u

            def copy_us(pr):
                us = sb.tile([128, 1024], F32R, tag="us", name="us", bufs=3)
                nc.scalar.copy(us[:], st[pr, "u"][:])
                st[pr, "us"] = us

            def trans_u(pr):
                uT = psA.tile([128, 1024], F32R, tag="a", name="uT")
                transpose_plane(uT[:], st[pr, "us"][:], IdR)
                st[pr, "uT"] = uT

            def copy_uTs(pr):
                # split the f32r copy across scalar and DVE to balance load
                uTs = sb.tile([128, 1024], F32R, tag="uTs", name="uTs", bufs=3)
                uT = st[pr, "uT"]
                nc.scalar.copy(uTs[:, 0:512], uT[:, 0:512])
                nc.vector.tensor_copy(uTs[:, 512:1024], uT[:, 512:1024])
                st[pr, "uTs"] = uTs

            def mob_s2(pr):
                uTs = st[pr, "uTs"]
                o = psA.tile([128, 1024], F32, tag="a", name="o")
                stage(o[:], BT, nBT, [uTs[:, 0:512]], [uTs[:, 512:1024]])
                osb = sb.tile([128, 1024], BF16, tag="osb", name="osb")
                nc.vector.tensor_copy(osb[:], o[:])
                nc.sync.dma_start(O[pr], osb[:])

            # Flat skewed software pipeline: pair pr runs stage at position p
            # during slot t = pr + p. Each slot emits one stage of ~18
            # different pairs, later stages first, so every engine's queue
            # interleaves many pairs and per-pair dependency chains never
            # stall the PE (which also keeps it at the 2.4 GHz p-state).
            stages = [
                (0, lambda pr: dma_in(pr, 0)),
                (3, lambda pr: zeta_s1(pr, 0)),
                (4, lambda pr: copy_ys(pr, 0)),
                (5, lambda pr: (trans_y(pr, 0), dma_in(pr, 1))),
                (6, lambda pr: copy_yTs(pr, 0)),
                (7, lambda pr: zeta_s2(pr, 0)),
                (8, lambda pr: zeta_s1(pr, 1)),
                (9, lambda pr: copy_ys(pr, 1)),
                (10, lambda pr: trans_y(pr, 1)),
                (11, lambda pr: copy_yTs(pr, 1)),
                (12, lambda pr: zeta_s2(pr, 1)),
                (13, lambda pr: mob_s1(pr)),
                (14, lambda pr: copy_us(pr)),
                (15, lambda pr: trans_u(pr)),
                (16, lambda pr: copy_uTs(pr)),
                (17, lambda pr: mob_s2(pr)),
            ]
            stages.sort(key=lambda s: -s[0])
            LAST = stages[0][0]
            for t in range(PAIRS + LAST):
                for pos, fn in stages:
                    pr = t - pos
                    if 0 <= pr < PAIRS:
                        fn(pr)

    nc.compile()
    return nc


@functools.lru_cache(maxsize=1)
def _get_nc():
    return _build()


def _host_in(M):
    """M [512, 2, 65536] f32 -> per-core Mi [PAIRS, 2, 128, 1024] bf16 contiguous.
    index16 = I*2^15 + p*2^8 + J*2^7 + l ; f-order (b, I, J, l)."""
    import ml_dtypes
    M6 = np.asarray(M, dtype=np.float32).reshape(NCORES, PAIRS, 2, 2, 2, 128, 2, 128)
    #                                      core, pair, b,  ch, I,  p,   J,  l
    Mi = np.ascontiguousarray(M6.transpose(0, 1, 3, 5, 2, 4, 6, 7))
    #                                      core, pair, ch, p, b, I, J, l
    return Mi.reshape(NCORES, PAIRS, 2, 128, 1024).astype(ml_dtypes.bfloat16)


def _host_out(Os):
    """Os list of [PAIRS, 128, 1024] bf16 per core -> [512, 65536, 1, 1] f32.
    o f-layout (I'', b, J, l)."""
    O = np.stack(Os).astype(np.float32).reshape(NCORES, PAIRS, 128, 2, 2, 2, 128)
    #                                            core, pair, p, I, b, J, l
    out = np.ascontiguousarray(O.transpose(0, 1, 4, 3, 2, 5, 6))
    #                                      core, pair, b, I, p, J, l
    return out.reshape(BATCH, L, 1, 1)


def _run(M, trace=False):
    import ml_dtypes
    from concourse.bass_utils import run_bass_kernel_spmd
    nc = _get_nc()
    AT7, BT7 = _constants()
    Cb = np.concatenate([AT7, np.eye(128, dtype=np.float32)],
                        axis=1).astype(ml_dtypes.bfloat16)
    Cr = np.concatenate([BT7, -BT7, np.eye(128, dtype=np.float32)], axis=1)
    Mi = _host_in(M)
    in_maps = [{"Mi": Mi[k], "Cb": Cb, "Cr": Cr} for k in range(NCORES)]
    res = run_bass_kernel_spmd(nc, in_maps, list(range(NCORES)), trace=trace)
    out = _host_out([res.results[k]["O"] for k in range(NCORES)])
    return out, res


def kernel(M):
    try:
        out, _ = _run(M, trace=False)
    except Exception:
        # one retry: a cold first execute has been observed to flake
        # (NRT_EXEC_UNIT_UNRECOVERABLE) and recover on rerun
        out, _ = _run(M, trace=False)
    return out


# revision 17
# speedup vs baseline: 2.6090x; 1.2496x over previous
"""TRN2 Bass kernel for nn_CNNDSTv2_batch: out = mobius16(zeta16(M[:,0]) * zeta16(M[:,1])).

Math: the 16-bit superset-zeta factorizes as Z = A8 @ X @ A8^T on the 256x256
view X[hi_byte, lo_byte]; A8 = [[A7, A7], [0, A7]] block-triangular, so each
8-bit stage is 3 accumulating 128x128 matmuls reusing one stationary. Each
two-sided transform runs as [stage, transpose, stage] and yields the transposed
result; chaining zeta -> multiply -> mobius lands back in natural layout.

Precision (modeled in numpy against f64, gate 2e-2, model says 3.9e-3):
- zeta path runs fully in bf16 (inputs are positive and the zeta->product->
  mobius composition is a positive map, so input/mid roundings stay relative);
- q (the commonality product feeding Mobius) keeps a 2-term f32r hi/lo split -
  rounding q is amplified ~100x by Mobius cancellation and dominates the
  error budget if single-rounded;
- the mobius mid-plane u is single-rounded f32r; the output is written bf16
  (final rounding, unamplified).

Perf: bf16 halves input DMA, y-plane copy bytes (2x DVE rate) and transpose
cost (1.0 cyc/row), and bf16 transpose PSUM tiles are 1 bank. walrus
enable-ldw-opt stays OFF (it miscompiles bf16/fp32 is_transpose); the tile
layer's own hoisted InstLdweights split gives overlapped weight loads anyway.
Emission is a flat skewed software pipeline (one pair per slot, 18 stage
positions) so the PE never waits on a same-pair dependency chain and holds
its 2.4 GHz p-state; copies are balanced across Activation/DVE/Pool.

Sharding: pure data parallel, batch 512 -> 64 per core across 8 cores.
"""
import sys
import os
import functools

sys.path.insert(0, "/opt/trn_rl_repo")
import numpy as np

BATCH = 512
L = 65536
NCORES = 8
BPC = BATCH // NCORES          # 64 batch elems per core
PAIRS = BPC // 2               # 2 elems per pipeline iteration


def _pc(v):
    return bin(v).count("1")


def _constants():
    k = np.arange(128)
    sup = (k[:, None] & k[None, :]) == k[None, :]          # sup[k,m] = k superset of m
    AT7 = sup.astype(np.float32)                           # lhsT for A7 @ x
    pc = np.array([_pc(i) for i in range(128)])
    sign = (-1.0) ** (pc[:, None] - pc[None, :])
    BT7 = (sup * sign).astype(np.float32)                  # lhsT for B7 @ x
    return AT7, BT7


def _build():
    import concourse.bacc as bacc
    import concourse.tile as tile
    import concourse.mybir as mybir

    # NOTE: walrus --enable-ldw-opt stays OFF: it miscompiles bf16 (and fp32)
    # is_transpose matmuls (verified: transposed planes come out as garbage).
    # The tile layer already pre-splits 2-byte-dtype matmuls into hoisted
    # InstLdweights + matmul, so bf16 weight loads overlap anyway; only the
    # f32r mobius matmuls pay a serial self-load.
    dt = mybir.dt
    F32, F32R, BF16 = dt.float32, dt.float32r, dt.bfloat16

    nc = bacc.Bacc("TRN2", target_bir_lowering=False, debug=False)

    # HBM layout (host pre-permuted, all DMAs contiguous):
    # Mi[pair, ch, p(=bits14..8), (b, I=bit15, J=bit7, l=bits6..0)] in bf16
    Mi = nc.dram_tensor("Mi", [PAIRS, 2, 128, 1024], BF16, kind="ExternalInput").ap()
    # Cb = [AT7 | Id] bf16 (exact 0/1), Cr = [BT7 | -BT7 | Id] f32r (exact 0/+-1)
    Cb_d = nc.dram_tensor("Cb", [128, 256], BF16, kind="ExternalInput").ap()
    Cr_d = nc.dram_tensor("Cr", [128, 384], F32R, kind="ExternalInput").ap()
    # O[pair, p, (I''=bit15, b, J=bit7, l=bits6..0)] bf16 - host unscrambles
    O = nc.dram_tensor("O", [PAIRS, 128, 1024], BF16, kind="ExternalOutput").ap()

    with tile.TileContext(nc) as tc:
        with tc.tile_pool(name="const", bufs=1) as cp, \
             tc.tile_pool(name="sbuf", bufs=2) as sb, \
             tc.tile_pool(name="psA", bufs=3, space="PSUM") as psA:
            Cb = cp.tile([128, 256], BF16, tag="Cb")
            nc.sync.dma_start(Cb[:], Cb_d)
            Cr = cp.tile([128, 384], F32R, tag="Cr")
            nc.sync.dma_start(Cr[:], Cr_d)
            ATb = Cb[:, 0:128]
            Idb = Cb[:, 128:256]
            BT = Cr[:, 0:128]
            nBT = Cr[:, 128:256]
            IdR = Cr[:, 256:384]

            def mm(out_ap, lhsT, rhs, start, stop):
                nc.tensor.matmul(out_ap, lhsT, rhs, start=start, stop=stop)

            def stage(dst, M, Mn, s0, s1):
                """dst[:, :512] = M@s0 + Mn@s1 ; dst[:, 512:] = M@s1.
                s0/s1: lists of 1-2 moving APs."""
                d1 = dst[:, 512:1024]
                for i, a in enumerate(s1):
                    mm(d1, M, a, start=(i == 0), stop=(i == len(s1) - 1))
                d0 = dst[:, 0:512]
                for i, a in enumerate(s0):
                    mm(d0, M, a, start=(i == 0), stop=False)
                for i, a in enumerate(s1):
                    mm(d0, Mn, a, start=False, stop=(i == len(s1) - 1))

            def transpose_plane(dst, src, Id):
                """dst[:, Jd*512 + b*256 + K*128 +: 128] =
                   src[:, K*512 + b*256 + Jd*128 +: 128].T  for Jd,b,K in {0,1}.
                One start/stop group per 512-wide half."""
                for Jd in (0, 1):
                    k = 0
                    for b in (0, 1):
                        for K in (0, 1):
                            nc.tensor.matmul(
                                dst[:, Jd * 512 + b * 256 + K * 128:][:, :128],
                                src[:, K * 512 + b * 256 + Jd * 128:][:, :128],
                                Id, is_transpose=True,
                                start=(k == 0), stop=(k == 3))
                            k += 1

            # --- software-pipelined emission: 2 pairs interleaved ---
            st = {}

            def dma_in(pr, c):
                xin = sb.tile([128, 1024], BF16, tag=f"xin{c}", bufs=4,
                              name=f"xin{c}")
                nc.sync.dma_start(xin[:], Mi[pr, c])
                st[pr, c, "x"] = xin

            def zeta_s1(pr, c):
                xr = st[pr, c, "x"][:].rearrange("p (b i f) -> p b i f", b=2, i=2)
                y = psA.tile([128, 1024], F32, tag="a", name="y")
                stage(y[:], ATb, ATb, [xr[:, :, 0]], [xr[:, :, 1]])
                st[pr, c, "y"] = y

            def copy_ys(pr, c):
                # PSUM f32 -> SBUF bf16, single rounding on the zeta path
                y = st[pr, c, "y"]
                ys = sb.tile([128, 1024], BF16, tag=f"ys{c}", name=f"ys{c}", bufs=3)
                nc.scalar.copy(ys[:], y[:])
                st[pr, c, "ys"] = ys

            def trans_y(pr, c):
                yT = psA.tile([128, 1024], BF16, tag="pT", bufs=2, name="yT")
                transpose_plane(yT[:], st[pr, c, "ys"][:], Idb)
                st[pr, c, "yT"] = yT

            def copy_yTs(pr, c):
                yTs = sb.tile([128, 1024], BF16, tag=f"yTs{c}", name=f"yTs{c}", bufs=3)
                nc.vector.tensor_copy(yTs[:], st[pr, c, "yT"][:])
                st[pr, c, "Ts"] = yTs

            def zeta_s2(pr, c):
                yTs = st[pr, c, "Ts"]
                z = psA.tile([128, 1024], F32, tag="a", name="z")
                stage(z[:], ATb, ATb, [yTs[:, 0:512]], [yTs[:, 512:1024]])
                if c == 0:
                    # z0s lives pos7 -> pos12 in the skew: deep ring
                    z0s = sb.tile([128, 1024], F32, tag="z0s", name="z0s", bufs=7)
                    nc.scalar.copy(z0s[:], z[:])
                    st[pr, "z0s"] = z0s
                else:
                    qf = sb.tile([128, 1024], F32, tag="qf", name="qf", bufs=3)
                    nc.vector.tensor_mul(qf[:], z[:], st[pr, "z0s"][:])
                    # q hi/lo split on the fast engines: Pool is too slow for
                    # this chain (its 3.6us CAST stalled the PE every slot)
                    qh = sb.tile([128, 1024], F32R, tag="qh", name="qh", bufs=3)
                    nc.scalar.copy(qh[:], qf[:])
                    ql = sb.tile([128, 1024], F32R, tag="ql", name="ql", bufs=3)
                    nc.vector.tensor_sub(ql[:], qf[:], qh[:].bitcast(F32))
                    st[pr, "q"] = (qh, ql)

            def mob_s1(pr):
                # like stage(), but all qh-dependent matmuls are issued before
                # the ql-dependent ones: ql is one DVE op behind qh, so this
                # gives the PE ~0.6us of ready work while ql lands
                qh, ql = st[pr, "q"]
                u = psA.tile([128, 1024], F32, tag="a", name="u")
                d0, d1 = u[:, 0:512], u[:, 512:1024]
                mm(d1, BT, qh[:, 512:1024], start=True, stop=False)
                mm(d0, BT, qh[:, 0:512], start=True, stop=False)
                mm(d0, nBT, qh[:, 512:1024], start=False, stop=False)
                mm(d0, BT, ql[:, 0:512], start=False, stop=False)
                mm(d0, nBT, ql[:, 512:1024], start=False, stop=True)
                mm(d1, BT, ql[:, 512:1024], start=False, stop=True)
                st[pr, "u"] = u

            def copy_us(pr):
                us = sb.tile([128, 1024], F32R, tag="us", name="us", bufs=3)
                nc.scalar.copy(us[:], st[pr, "u"][:])
                st[pr, "us"] = us

            def trans_u(pr):
                uT = psA.tile([128, 1024], F32R, tag="a", name="uT")
                transpose_plane(uT[:], st[pr, "us"][:], IdR)
                st[pr, "uT"] = uT

            def copy_uTs(pr):
                uTs = sb.tile([128, 1024], F32R, tag="uTs", name="uTs", bufs=3)
                nc.vector.tensor_copy(uTs[:], st[pr, "uT"][:])
                st[pr, "uTs"] = uTs

            def mob_s2(pr):
                uTs = st[pr, "uTs"]
                o = psA.tile([128, 1024], F32, tag="a", name="o")
                stage(o[:], BT, nBT, [uTs[:, 0:512]], [uTs[:, 512:1024]])
                osb = sb.tile([128, 1024], BF16, tag="osb", name="osb")
                nc.vector.tensor_copy(osb[:], o[:])
                nc.sync.dma_start(O[pr], osb[:])

            # Flat skewed software pipeline: pair pr runs stage at position p
            # during slot t = pr + p. Each slot emits one stage of ~18
            # different pairs, later stages first, so every engine's queue
            # interleaves many pairs and per-pair dependency chains never
            # stall the PE (which also keeps it at the 2.4 GHz p-state).
            stages = [
                (0, lambda pr: dma_in(pr, 0)),
                (3, lambda pr: zeta_s1(pr, 0)),
                (4, lambda pr: copy_ys(pr, 0)),
                (5, lambda pr: (trans_y(pr, 0), dma_in(pr, 1))),
                (6, lambda pr: copy_yTs(pr, 0)),
                (7, lambda pr: zeta_s2(pr, 0)),
                (8, lambda pr: zeta_s1(pr, 1)),
                (9, lambda pr: copy_ys(pr, 1)),
                (10, lambda pr: trans_y(pr, 1)),
                (11, lambda pr: copy_yTs(pr, 1)),
                (12, lambda pr: zeta_s2(pr, 1)),
                (13, lambda pr: mob_s1(pr)),
                (14, lambda pr: copy_us(pr)),
                (15, lambda pr: trans_u(pr)),
                (16, lambda pr: copy_uTs(pr)),
                (17, lambda pr: mob_s2(pr)),
            ]
            stages.sort(key=lambda s: -s[0])
            LAST = stages[0][0]
            for t in range(PAIRS + LAST):
                for pos, fn in stages:
                    pr = t - pos
                    if 0 <= pr < PAIRS:
                        fn(pr)

    nc.compile()
    return nc


@functools.lru_cache(maxsize=1)
def _get_nc():
    return _build()


def _host_in(M):
    """M [512, 2, 65536] f32 -> per-core Mi [PAIRS, 2, 128, 1024] bf16 contiguous.
    index16 = I*2^15 + p*2^8 + J*2^7 + l ; f-order (b, I, J, l)."""
    import ml_dtypes
    M6 = np.asarray(M, dtype=np.float32).reshape(NCORES, PAIRS, 2, 2, 2, 128, 2, 128)
    #                                      core, pair, b,  ch, I,  p,   J,  l
    Mi = np.ascontiguousarray(M6.transpose(0, 1, 3, 5, 2, 4, 6, 7))
    #                                      core, pair, ch, p, b, I, J, l
    return Mi.reshape(NCORES, PAIRS, 2, 128, 1024).astype(ml_dtypes.bfloat16)


def _host_out(Os):
    """Os list of [PAIRS, 128, 1024] bf16 per core -> [512, 65536, 1, 1] f32.
    o f-layout (I'', b, J, l)."""
    O = np.stack(Os).astype(np.float32).reshape(NCORES, PAIRS, 128, 2, 2, 2, 128)
    #                                            core, pair, p, I, b, J, l
    out = np.ascontiguousarray(O.transpose(0, 1, 4, 3, 2, 5, 6))
    #                                      core, pair, b, I, p, J, l
    return out.reshape(BATCH, L, 1, 1)


def _run(M, trace=False):
    import ml_dtypes
    from concourse.bass_utils import run_bass_kernel_spmd
    nc = _get_nc()
    AT7, BT7 = _constants()
    Cb = np.concatenate([AT7, np.eye(128, dtype=np.float32)],
                        axis=1).astype(ml_dtypes.bfloat16)
    Cr = np.concatenate([BT7, -BT7, np.eye(128, dtype=np.float32)], axis=1)
    Mi = _host_in(M)
    in_maps = [{"Mi": Mi[k], "Cb": Cb, "Cr": Cr} for k in range(NCORES)]
    res = run_bass_kernel_spmd(nc, in_maps, list(range(NCORES)), trace=trace)
    out = _host_out([res.results[k]["O"] for k in range(NCORES)])
    return out, res


def kernel(M):
    try:
        out, _ = _run(M, trace=False)
    except Exception:
        # one retry: a cold first execute has been observed to flake
        # (NRT_EXEC_UNIT_UNRECOVERABLE) and recover on rerun
        out, _ = _run(M, trace=False)
    return out
